# revision 1
# baseline (speedup 1.0000x reference)
"""KAN-FNO block on 8 Trainium2 NeuronCores — hand-written Bass/Tile kernel.

Strategy: data-parallel over batch (16 -> 2 per core), weights replicated.
Spectral conv = truncated DFT via PE matmuls (stationary DFT matrices) with
PE transposes for layout changes; per-frequency channel mix = 512 small
matmuls with block-complex weights; KAN layers use the cardinal B-spline
basis computed by 1 ScalarE |.| op + 2 custom DVE ops per dual-basis pair
(numerically safe in bf16), contracted on the PE.

kernel(**inputs) takes full fp32 inputs, returns the full fp32 output.
"""
import numpy as np
import ml_dtypes

import concourse.bass as bass
import concourse.bacc as bacc
import concourse.mybir as mybir
from concourse.tile import TileContext
from concourse import bass_utils
from concourse import dve_ops
from concourse.dve_spec import Spec, Src0, Src1, C0, C2, One, relu, sq, lower, _has_src1
from concourse.dve_uop import DveOpSpec

BF16 = mybir.dt.bfloat16
F32 = mybir.dt.float32

B, C, H, W = 16, 64, 128, 128
NCORES = 8
BLOC = B // NCORES          # 2 samples per core
MODES = 16                  # kept w-freqs
R32 = 32                    # kept h-freqs (16 low + 16 high)
HKNOT = 0.4
GRID0 = -2.2                # first knot
PIX = H * W                 # 16384 pixels per sample
CHUNK = 1024                # KAN pixel chunk (8 h-rows)
NCH = PIX // CHUNK          # chunks per sample
HROWS = CHUNK // W          # h-rows per chunk

# ---------------------------------------------------------------- custom DVE
_REGISTERED = {}


def _register_dve():
    if _REGISTERED:
        return _REGISTERED

    def mk(name, spec):
        row = max(dve_ops._SUB_OPCODE_FOR_NAME.values()) + 1
        assert row < 0x20
        dve_ops._SUB_OPCODE_FOR_NAME[name] = row
        op = dve_ops.DveOp(name, spec, subdim=False, uops_sha={})
        for ver in ("v3", "v4"):
            try:
                r = DveOpSpec(name=name, opcode=row, uops=lower(spec, ver=ver),
                              rd1_en=_has_src1(spec))
                op.uops_sha[ver] = r.sha(ver)
            except Exception:
                pass
        dve_ops.OPS.append(op)
        dve_ops.CUSTOM_DVE_SPECS[name] = spec
        return op

    # u = relu(2 - A);  ALPHA: uc = u^3
    _u = relu(C2 - Src0)
    alpha = Spec(
        body=_u * sq(_u),
        reference=lambda in0, in1, s0, s1, imm2:
            (np.maximum(imm2 - in0, 0.0) ** 3).astype(np.float32),
    )
    # BETA: out = uc + (-4) * relu(u - 1)^3   (= u^3 - 4(u-1)+^3 = 6*B)
    _v = relu(relu(C2 - Src0) - One)
    beta = Spec(
        body=Src1 + (_v * sq(_v)) * C0,
        reference=lambda in0, in1, s0, s1, imm2:
            (in1 + s0 * np.maximum(np.maximum(imm2 - in0, 0.0) - 1.0, 0.0) ** 3
             ).astype(np.float32),
    )
    _REGISTERED['alpha'] = mk("KANFNO_ALPHA", alpha)
    _REGISTERED['beta'] = mk("KANFNO_BETA", beta)
    return _REGISTERED


# ---------------------------------------------------------------- host consts
def _np_consts():
    r_idx = np.concatenate([np.arange(MODES), np.arange(H - MODES, H)])
    h = np.arange(H)
    th = 2 * np.pi * np.outer(r_idx, h) / H            # (32,128)
    AhT = np.zeros((H, 64), np.float32)
    AhT[:, :32] = np.cos(th).T
    AhT[:, 32:] = np.sin(th).T
    w = np.arange(W)
    k = np.arange(MODES)
    ph = 2 * np.pi * np.outer(k, w) / W                # (16,128)
    Fw = np.zeros((W, 32), np.float32)
    Fw[:, :16] = np.cos(ph).T
    Fw[:, 16:] = np.sin(ph).T
    gam = np.full(MODES, 2.0 / (H * W))
    gam[0] = 1.0 / (H * W)
    G1 = (np.cos(ph) * gam[:, None]).astype(np.float32)   # (16,128)
    G2 = (np.sin(ph) * gam[:, None]).astype(np.float32)
    AhI = np.zeros((64, H), np.float32)
    AhI[:32] = np.cos(th)
    AhI[32:] = -np.sin(th)
    # ACT |.| bias for 4 dual pairs: rows 0:64 -> basis 2p, 64:128 -> 2p+1
    biasv = np.zeros((128, 4), np.float32)
    for p in range(4):
        for half in range(2):
            j = 2 * p + half
            gj = GRID0 + HKNOT * j
            biasv[half * 64:(half + 1) * 64, p] = -(gj / HKNOT + 2.0)
    ident = np.eye(128, dtype=np.float32)
    return AhT, Fw, G1, G2, AhI, biasv, ident


def _bf16(a):
    return np.asarray(a, dtype=ml_dtypes.bfloat16)


def _host_prep(x, spec_w1_r, spec_w1_i, spec_w2_r, spec_w2_i, conv_w, conv_b,
               k1_base, k1_spline, k1_scaler, k2_base, k2_spline, k2_scaler):
    """Build the per-core input maps (core-invariant weights + per-core x)."""
    AhT, Fw, G1, G2, AhI, biasv, ident = _np_consts()
    # mix blocks -> (32, 128, 16, 128) [r, kappa, k, m] for clean DMA
    Wr = np.concatenate([spec_w1_r, spec_w2_r], axis=2)  # (i,o,32,16)
    Wi = np.concatenate([spec_w1_i, spec_w2_i], axis=2)
    blk = np.empty((R32, 128, MODES, 128), np.float32)  # reshaped below
    # rows kappa = [Xr_i; Xi'_i], cols m = [Yr_o; Yi_o]
    # Yr = Wr X_r + Wi Xi' ; Yi = Wi X_r - Wr Xi'
    blk[:, :64, :, :64] = Wr.transpose(2, 0, 3, 1)       # (r,i,k,o)
    blk[:, 64:, :, :64] = Wi.transpose(2, 0, 3, 1)
    blk[:, :64, :, 64:] = Wi.transpose(2, 0, 3, 1)
    blk[:, 64:, :, 64:] = -Wr.transpose(2, 0, 3, 1)

    def kan_w(base_w, spline_w, scaler):
        Wm = (spline_w * scaler[..., None]) / 6.0        # (o, i, 8)
        wk = np.empty((4, 128, 64), np.float32)
        for kc in range(4):
            wk[kc, :64] = Wm[:, :, 2 * kc].T             # (i, o)
            wk[kc, 64:] = Wm[:, :, 2 * kc + 1].T
        return wk, np.ascontiguousarray(base_w.T)

    k1w, k1b = kan_w(k1_base, k1_spline, k1_scaler)
    k2w, k2b = kan_w(k2_base, k2_spline, k2_scaler)

    shared = {
        'aht': _bf16(AhT), 'fw': _bf16(Fw),
        'g1': _bf16(G1), 'g2': _bf16(G2), 'ahi': _bf16(AhI),
        'wblk': _bf16(blk.reshape(R32, 128, MODES * 128)),
        'convw': _bf16(conv_w.T.copy()),
        'convb': conv_b.reshape(64, 1).astype(np.float32),
        'k1w': _bf16(k1w), 'k1b': _bf16(k1b),
        'k2w': _bf16(k2w), 'k2b': _bf16(k2b),
        'biasv': biasv.astype(np.float32), 'ident': _bf16(ident),
    }
    xb = _bf16(x).reshape(NCORES, BLOC * C, H, W)
    in_maps = []
    for core in range(NCORES):
        m = dict(shared)
        m['x'] = np.ascontiguousarray(xb[core])
        in_maps.append(m)
    return in_maps


_DEBUG_SKIP = set()  # {'quad','i1c','zadd'} - replace TT groups with copies


# ---------------------------------------------------------------- bass build
def build_nc():
    ops = _register_dve()
    nc = bacc.Bacc("TRN2", target_bir_lowering=False, debug=False)

    x_in = nc.dram_tensor("x", [BLOC * C, H, W], BF16, kind="ExternalInput")
    aht = nc.dram_tensor("aht", [H, 64], BF16, kind="ExternalInput")
    fw = nc.dram_tensor("fw", [W, 32], BF16, kind="ExternalInput")
    g1 = nc.dram_tensor("g1", [MODES, W], BF16, kind="ExternalInput")
    g2 = nc.dram_tensor("g2", [MODES, W], BF16, kind="ExternalInput")
    ahi = nc.dram_tensor("ahi", [64, H], BF16, kind="ExternalInput")
    wblk = nc.dram_tensor("wblk", [R32, 128, MODES * 128], BF16,
                          kind="ExternalInput")
    convw = nc.dram_tensor("convw", [C, C], BF16, kind="ExternalInput")
    convb = nc.dram_tensor("convb", [C, 1], F32, kind="ExternalInput")
    k1w = nc.dram_tensor("k1w", [4, 128, 64], BF16, kind="ExternalInput")
    k1b = nc.dram_tensor("k1b", [C, C], BF16, kind="ExternalInput")
    k2w = nc.dram_tensor("k2w", [4, 128, 64], BF16, kind="ExternalInput")
    k2b = nc.dram_tensor("k2b", [C, C], BF16, kind="ExternalInput")
    biasv = nc.dram_tensor("biasv", [128, 4], F32, kind="ExternalInput")
    ident = nc.dram_tensor("ident", [128, 128], BF16, kind="ExternalInput")
    out = nc.dram_tensor("out", [BLOC * C, PIX], BF16, kind="ExternalOutput")

    AF = mybir.ActivationFunctionType

    def cp(dst, src):
        # pin PSUM->SBUF copies to ScalarE so consumers wait on one engine
        nc.scalar.activation(dst, src, AF.Copy)

    with TileContext(nc) as tc:
        with tc.tile_pool(name="consts", bufs=1) as pconst, \
                tc.tile_pool(name="zs_pool", bufs=1) as pzs:
            aht_s = pconst.tile([H, 64], BF16)
            nc.sync.dma_start(aht_s[:], aht[:, :])
            fw_s = pconst.tile([W, 32], BF16)
            nc.sync.dma_start(fw_s[:], fw[:, :])
            g1_s = pconst.tile([MODES, W], BF16)
            nc.sync.dma_start(g1_s[:], g1[:, :])
            g2_s = pconst.tile([MODES, W], BF16)
            nc.sync.dma_start(g2_s[:], g2[:, :])
            ahi_s = pconst.tile([64, H], BF16)
            nc.sync.dma_start(ahi_s[:], ahi[:, :])
            convw_s = pconst.tile([C, C], BF16)
            nc.sync.dma_start(convw_s[:], convw[:, :])
            convb_s = pconst.tile([C, 1], F32)
            nc.sync.dma_start(convb_s[:], convb[:, :])
            biasv_s = pconst.tile([128, 4], F32)
            nc.sync.dma_start(biasv_s[:], biasv[:, :])
            ident_s = pconst.tile([128, 128], BF16)
            nc.sync.dma_start(ident_s[:], ident[:, :])
            kan_w = []
            for t, (wt, bt) in enumerate(((k1w, k1b), (k2w, k2b))):
                ws = pconst.tile([128, 4 * 64], BF16, name=f"kanw{t}")
                for kc in range(4):
                    nc.sync.dma_start(ws[:, kc * 64:(kc + 1) * 64],
                                      wt[kc, :, :])
                bs = pconst.tile([C, C], BF16, name=f"kanb{t}")
                nc.sync.dma_start(bs[:], bt[:, :])
                kan_w.append((ws, bs))

            zs = [pzs.tile([128, C * H], BF16, name=f"zs{b_}")
                  for b_ in range(BLOC)]

            # ================= spectral branch =================
            with tc.tile_pool(name="sb1", bufs=1) as ps1:
                # ---- stage A (h-DFT): t1[w, bc*64+rho] ----
                t1 = ps1.tile([W, BLOC * C * 64], BF16, name="t1")
                with tc.tile_pool(name="pxa", bufs=1) as pxa:
                    xa = pxa.tile([H, BLOC * C * W], BF16)
                    nc.sync.dma_start(
                        xa[:], x_in.ap().rearrange("bc h w -> h bc w"))
                    with tc.tile_pool(name="pA", bufs=4, space="PSUM") as pa:
                        for bc in range(BLOC * C):
                            p1 = pa.tile([W, 64], F32, tag="p1")
                            nc.tensor.matmul(p1[:],
                                             xa[:, bc * W:(bc + 1) * W],
                                             aht_s[:], start=True, stop=True)
                            cp(t1[:, bc * 64:(bc + 1) * 64], p1[:])

                # ---- stage B (w-DFT) + quadrant combine ----
                qr = [ps1.tile([16, 2048], BF16, name=f"qr{b_}")
                      for b_ in range(BLOC)]
                qi = [ps1.tile([16, 2048], BF16, name=f"qi{b_}")
                      for b_ in range(BLOC)]
                with tc.tile_pool(name="pB", bufs=1, space="PSUM") as pb:
                    # p2s: rows b0@0, b1@32; free [kc block 0:4096 | ks 4096:]
                    p2s = ps1.tile([64, 8192], BF16, name="p2s")
                    for half, fsl in enumerate((slice(0, 16), slice(16, 32))):
                        p2 = pb.tile([64, 4096], F32, tag="p2")
                        for bc in range(BLOC * C):
                            b_, c_ = divmod(bc, C)
                            nc.tensor.matmul(
                                p2[32 * b_:32 * b_ + 16,
                                   64 * c_:64 * c_ + 64],
                                fw_s[:, fsl], t1[:, bc * 64:(bc + 1) * 64],
                                start=True, stop=True)
                        for b_ in range(BLOC):
                            cp(p2s[32 * b_:32 * b_ + 16,
                                   4096 * half:4096 * half + 4096],
                               p2[32 * b_:32 * b_ + 16, :])
                    p2v = p2s[:].rearrange("p (h c r) -> p h c r", h=2, c=64)
                    for b_ in range(BLOC):
                        o_r = qr[b_][:].rearrange("p (r c) -> p c r", c=64)
                        o_i = qi[b_][:].rearrange("p (r c) -> p c r", c=64)
                        sl = slice(32 * b_, 32 * b_ + 16)
                        nc.vector.tensor_sub(
                            o_r, p2v[sl, 0, :, 0:32], p2v[sl, 1, :, 32:64])
                        nc.vector.tensor_add(
                            o_i, p2v[sl, 0, :, 32:64], p2v[sl, 1, :, 0:32])

            # ---- mix-input transposes -> M_r ----
                m_tiles = []
                with tc.tile_pool(name="pM", bufs=4, space="PSUM") as pm:
                    for r in range(R32):
                        mp = pm.tile([128, 2 * MODES], BF16, tag="mp")
                        for b_ in range(BLOC):
                            nc.tensor.matmul(
                                mp[0:64, b_ * MODES:(b_ + 1) * MODES],
                                qr[b_][:, r * 64:(r + 1) * 64],
                                ident_s[0:16, 0:16], is_transpose=True,
                                start=True, stop=True)
                            nc.tensor.matmul(
                                mp[64:128, b_ * MODES:(b_ + 1) * MODES],
                                qi[b_][:, r * 64:(r + 1) * 64],
                                ident_s[0:16, 0:16], is_transpose=True,
                                start=True, stop=True)
                        ms = ps1.tile([128, 2 * MODES], BF16, name=f"m{r}")
                        cp(ms[:], mp[:])
                        m_tiles.append(ms)

                # ---- channel mix: Y layout cols = k*2 + b ----
                ys_tiles = []
                with tc.tile_pool(name="wbuf", bufs=4) as pw, \
                        tc.tile_pool(name="pY", bufs=4, space="PSUM") as py:
                    for r in range(R32):
                        wb = pw.tile([128, MODES * 128], BF16, tag="wb")
                        nc.sync.dma_start(wb[:], wblk[r, :, :])
                        yp = py.tile([128, 2 * MODES], F32, tag="yp")
                        mv = m_tiles[r][:].rearrange("p (b k) -> p k b",
                                                     k=MODES)
                        for kk in range(MODES):
                            nc.tensor.matmul(
                                yp[:, kk * 2:kk * 2 + 2],
                                wb[:, kk * 128:(kk + 1) * 128],
                                mv[:, kk, :], start=True, stop=True)
                        ysr = ps1.tile([128, 2 * MODES], BF16, name=f"ys{r}")
                        cp(ysr[:], yp[:])
                        ys_tiles.append(ysr)

                # ---- Y transposes (8 pairs per tile, packed in free) ----
                yt_tiles = []
                with tc.tile_pool(name="pYT", bufs=2, space="PSUM") as pyt:
                    for g in range(8):
                        ytp = pyt.tile([16, 8 * 128], BF16, tag="ytp")
                        for s in range(8):
                            b_, r = divmod(g * 8 + s, R32)
                            ysv = ys_tiles[r][:].rearrange(
                                "p (k b) -> p b k", b=BLOC)
                            nc.tensor.matmul(
                                ytp[:, s * 128:(s + 1) * 128],
                                ysv[:, b_, :], ident_s[:, :],
                                is_transpose=True, start=True, stop=True)
                        yts = ps1.tile([16, 8 * 128], BF16, name=f"yt{g}")
                        cp(yts[:], ytp[:])
                        yt_tiles.append(yts)

                # ---- I1 + combine -> Abr/Abi ----
                ab_r = [ps1.tile([128, 2048], BF16, name=f"abr{b_}")
                        for b_ in range(BLOC)]
                ab_i = [ps1.tile([128, 2048], BF16, name=f"abi{b_}")
                        for b_ in range(BLOC)]
                with tc.tile_pool(name="pI1", bufs=3, space="PSUM") as pi1, \
                    tc.tile_pool(name="pI1s", bufs=3) as pi1s:
                    for g in range(8):
                        for s in range(8):
                            b_, r = divmod(g * 8 + s, R32)
                            o1 = pi1.tile([128, 128], F32, tag="o1")
                            o2 = pi1.tile([128, 128], F32, tag="o2")
                            yts = yt_tiles[g][:, s * 128:(s + 1) * 128]
                            nc.tensor.matmul(o1[:], g1_s[:], yts,
                                             start=True, stop=True)
                            nc.tensor.matmul(o2[:], g2_s[:], yts,
                                             start=True, stop=True)
                            o1s = pi1s.tile([128, 128], BF16, tag="o1s")
                            cp(o1s[:], o1[:])
                            nc.vector.tensor_sub(
                                ab_r[b_][:, r * 64:(r + 1) * 64],
                                o1s[:, 0:64], o2[:, 64:128])
                            nc.vector.tensor_add(
                                ab_i[b_][:, r * 64:(r + 1) * 64],
                                o2[:, 0:64], o1s[:, 64:128])

                # ---- As transposes + I2 -> zs ----
                with tc.tile_pool(name="pAS", bufs=3, space="PSUM") as pas, \
                        tc.tile_pool(name="as2p", bufs=4) as pas2, \
                        tc.tile_pool(name="pZ", bufs=4, space="PSUM") as pz:
                    for e in range(C):
                        asp = pas.tile([64, 256], BF16, tag="asp")
                        for half in range(2):
                            b_, o_ = divmod(e * 2 + half, C)
                            arv = ab_r[b_][:].rearrange("p (r o) -> p o r",
                                                        o=64)
                            aiv = ab_i[b_][:].rearrange("p (r o) -> p o r",
                                                        o=64)
                            nc.tensor.matmul(
                                asp[0:32, half * 128:(half + 1) * 128],
                                arv[:, o_, :], ident_s[:, :],
                                is_transpose=True, start=True, stop=True)
                            nc.tensor.matmul(
                                asp[32:64, half * 128:(half + 1) * 128],
                                aiv[:, o_, :], ident_s[:, :],
                                is_transpose=True, start=True, stop=True)
                        as2 = pas2.tile([64, 256], BF16, tag="as2")
                        cp(as2[:], asp[:])
                        for half in range(2):
                            b_, o_ = divmod(e * 2 + half, C)
                            zp = pz.tile([128, 128], F32, tag="zp")
                            nc.tensor.matmul(
                                zp[:], as2[:, half * 128:(half + 1) * 128],
                                ahi_s[:], start=True, stop=True)
                            cp(zs[b_][:, o_ * H:(o_ + 1) * H], zp[:])

            # ================= conv + z-gather + KAN =================
            with tc.tile_pool(name="pxc", bufs=3) as pxc, \
                    tc.tile_pool(name="kanp", bufs=2) as pk, \
                    tc.tile_pool(name="scr", bufs=2) as psc, \
                    tc.tile_pool(name="pYA", bufs=2, space="PSUM") as pya, \
                    tc.tile_pool(name="pKM", bufs=2, space="PSUM") as pkm:
                for b_ in range(BLOC):
                    zv = zs[b_][:].rearrange("p (o h) -> p h o", o=64)
                    for hg in range(NCH):
                        xct = pxc.tile([C, CHUNK], BF16, tag="xct")
                        nc.sync.dma_start(
                            xct[:].rearrange("c (h w) -> c h w", w=W),
                            x_in.ap().rearrange("(b c) h w -> b c h w",
                                                b=BLOC)
                            [b_, :, hg * HROWS:(hg + 1) * HROWS, :])
                        ypsum = pya.tile([64, CHUNK], F32, tag="ypsum")
                        for n in range(CHUNK // 512):
                            nc.tensor.matmul(
                                ypsum[:, n * 512:(n + 1) * 512],
                                convw_s[:],
                                xct[:, n * 512:(n + 1) * 512],
                                start=True, stop=True)
                        ztp = pya.tile([64, CHUNK], BF16, tag="ztp")
                        for hh in range(HROWS):
                            h_ = hg * HROWS + hh
                            nc.tensor.matmul(
                                ztp[:, hh * 128:(hh + 1) * 128],
                                zv[:, h_, :], ident_s[:, :],
                                is_transpose=True, start=True, stop=True)
                        ysb = pk.tile([64, CHUNK], F32, tag="ysb")
                        nc.scalar.activation(ysb[:], ypsum[:], AF.Identity,
                                             bias=convb_s[:, 0:1])
                        yfin = pk.tile([64, CHUNK], F32, tag="yfin")
                        if 'zadd' in _DEBUG_SKIP:
                            nc.vector.tensor_copy(yfin[:], ysb[:])
                        else:
                            nc.vector.tensor_add(yfin[:], ysb[:], ztp[:])
                        ycur = yfin
                        for lay in range(2):
                            ws, bs = kan_w[lay]
                            ys2 = psc.tile([128, CHUNK], F32, tag="ys2")
                            nc.vector.tensor_copy(ys2[0:64, :], ycur[:])
                            nc.vector.tensor_copy(ys2[64:128, :], ycur[:])
                            fsilu = psc.tile([64, CHUNK], BF16, tag="fsilu")
                            nc.scalar.activation(fsilu[:], ycur[:], AF.Silu)
                            fts = []
                            for p in range(4):
                                ap_t = psc.tile([128, CHUNK], F32, tag="ap")
                                nc.scalar.activation(
                                    ap_t[:], ys2[:], AF.Abs,
                                    bias=biasv_s[:, p:p + 1], scale=2.5)
                                uc_t = psc.tile([128, CHUNK], F32, tag="uc")
                                nc.vector._custom_dve(
                                    ops['alpha'], out=uc_t[:], in0=ap_t[:],
                                    imm2=2.0)
                                f_t = psc.tile([128, CHUNK], BF16,
                                               tag=f"f{p}")
                                nc.vector._custom_dve(
                                    ops['beta'], out=f_t[:], in0=ap_t[:],
                                    in1=uc_t[:], s0=-4.0, imm2=2.0)
                                fts.append(f_t)
                            is_last = lay == 1
                            if not is_last:
                                dst = pk.tile([64, CHUNK], F32, tag="ynext")
                            else:
                                dst = pk.tile([64, CHUNK], BF16, tag="obuf")
                            for n in range(CHUNK // 512):
                                kp = pkm.tile([64, 512], F32, tag="kp")
                                sl = slice(n * 512, (n + 1) * 512)
                                for kc in range(4):
                                    nc.tensor.matmul(
                                        kp[:], ws[:, kc * 64:(kc + 1) * 64],
                                        fts[kc][:, sl],
                                        start=(kc == 0), stop=False)
                                nc.tensor.matmul(kp[:], bs[:],
                                                 fsilu[:, sl],
                                                 start=False, stop=True)
                                if not is_last:
                                    nc.vector.tensor_copy(dst[:, sl], kp[:])
                                else:
                                    nc.scalar.activation(dst[:, sl], kp[:],
                                                         AF.Gelu)
                            ycur = dst
                        nc.sync.dma_start(
                            out.ap()[b_ * C:(b_ + 1) * C,
                                     hg * CHUNK:(hg + 1) * CHUNK],
                            ycur[:])
    nc.compile()
    return nc


# ---------------------------------------------------------------- runner
_NC = None


def _get_nc():
    global _NC
    if _NC is None:
        _NC = build_nc()
    return _NC


def kernel(x, spec_w1_r, spec_w1_i, spec_w2_r, spec_w2_i, conv_w, conv_b,
           k1_base, k1_spline, k1_scaler, k2_base, k2_spline, k2_scaler):
    nc = _get_nc()
    in_maps = _host_prep(x, spec_w1_r, spec_w1_i, spec_w2_r, spec_w2_i,
                         conv_w, conv_b, k1_base, k1_spline, k1_scaler,
                         k2_base, k2_spline, k2_scaler)
    res = bass_utils.run_bass_kernel_spmd(nc, in_maps,
                                          core_ids=list(range(NCORES)))
    outs = [res.results[c]['out'] for c in range(NCORES)]
    y = np.stack(outs).astype(np.float32)
    return y.reshape(B, C, H, W)

